# revision 4
# baseline (speedup 1.0000x reference)
"""W8A16 column-parallel linear for TRN2, 8 NeuronCores.

Computes y = x @ (qweight * w_scales).T + bias with
  x        [8, 1, 8192]  fp16
  qweight  [28672, 8192] int8 (per-row symmetric quant)
  w_scales [28672, 1]    fp16
  bias     [28672]       fp16
  y        [8, 1, 28672] fp16

Sharding: column-parallel — each of the 8 cores owns 3584 output rows
(qweight/w_scales/bias shard), x replicated. No collectives; outputs are
concatenated on the host.

Per-core kernel: stream the int8 weight shard from HBM in U-ktile groups
(host pre-permuted to k-major so each partition's group slice is one
contiguous 14KB run -> large DMA descriptors), convert int8->fp16
on-chip with a three-way free-dim split across VectorE (2x-port CAST),
ScalarE (activation Copy) and Pool/GpSimd (tensor_copy) so the converter
trio outruns the ~400GB/s DMA stream, then accumulate [8, 512]-chunk
PSUM regions with fp16 matmuls (stationary x^T tile, moving weight
tile). The 7 chunks are spread over 3 PE column groups (PSUM base
partition) so three moving streams run concurrently. Bias/scales enter
as out = (sum x*q + b/s) * s: a K=1 matmul of ones^T @ (b/s) opens each
PSUM accumulation group; tail scale-muls run on VectorE and Pool in
parallel. x/bias/scales ride the Activation-engine DGE queue so the sync-engine
HWDGE queue carries nothing but weights.
"""

import numpy as np

import concourse.bacc as bacc
import concourse.mybir as mybir
import concourse.tile as tile
from concourse.bass_utils import run_bass_kernel_spmd

B, S, K, N = 8, 1, 8192, 28672
M = B * S                 # 8 rows in the GEMM
NCORES = 8
NS = N // NCORES          # 3584 output rows per core
KT = K // 128             # 64 k-tiles
U = 4                     # k-tiles per DMA/conversion group
NCHUNK = NS // 512        # 7 psum chunks of 512

# three-way free-dim split of the int8->fp16 conversion:
# VectorE [0, DV), ScalarE [DV, DV+DA), Pool [DV+DA, NS)
DV = 1984
DA = 1216
DP = NS - DV - DA         # 384

_CACHE = {}

# chunk -> PE column-group (0,1,2 -> array cols 0-31/32-63/64-95). Three
# concurrent moving streams triple the PE's weight-streaming rate.
CHUNK_GRP = [0, 0, 0, 1, 1, 2, 2]
GRP_BASE = [32 * j for j in CHUNK_GRP]         # PSUM base partition per chunk
GRP_SPAN = {0: (0, 1536), 1: (1536, 2560), 2: (2560, 3584)}


def _build():
    nc = bacc.Bacc()
    xp = nc.declare_dram_parameter("x", [128, KT * M], mybir.dt.float16, isOutput=False)
    # k-major int8 weights: qp[p, kt*NS + n] = w[n, kt*128 + p]
    qp = nc.declare_dram_parameter("qt", [128, KT * NS], mybir.dt.int8, isOutput=False)
    sp = nc.declare_dram_parameter("s", [24, NS], mybir.dt.float16, isOutput=False)
    bp = nc.declare_dram_parameter("b", [1, NS], mybir.dt.float16, isOutput=False)
    op = nc.declare_dram_parameter("out", [M, NS], mybir.dt.float16, isOutput=True)

    # whole-param rearranges per group size: emits the efficient DMA
    # descriptor layout (one contiguous usz*NS run per partition)
    qru = {
        usz: qp.rearrange("p (g u n) -> g p u n", u=usz, n=NS)
        for usz in (1, 2, 4)
    }

    # uniform big groups keep the DMA stream (the binding resource) at full
    # efficiency; small groups only at the tail so the convert->matmul->
    # scale pipeline drains quickly after the last weight byte lands.
    GROUPS = [U] * 15 + [2, 1, 1]
    assert sum(GROUPS) == KT

    # per-ktile matmul issue order rotates through the PE column groups so
    # the three streams start back-to-back instead of blocking each other
    ISSUE = [0, 3, 5, 1, 4, 6, 2]

    with tile.TileContext(nc) as tc:
        with (
            tc.tile_pool(name="const", bufs=1) as constp,
            tc.tile_pool(name="wq", bufs=5) as wqp,
            tc.tile_pool(name="wf", bufs=3) as wfp,
            tc.tile_pool(name="psum", bufs=1, space="PSUM") as psp,
            tc.tile_pool(name="outp", bufs=1) as outp,
        ):
            xsb = constp.tile([128, KT * M], mybir.dt.float16, tag="xsb")
            sb = constp.tile([72, NS], mybir.dt.float16, tag="sb")
            b1 = constp.tile([1, NS], mybir.dt.float16, tag="b1")
            ones = constp.tile([1, M], mybir.dt.float16, tag="ones")

            # weight stream owns the sync-engine HWDGE queue exclusively;
            # constants ride the Activation-engine DGE queue in parallel
            wq0 = wqp.tile([128, GROUPS[0], NS], mybir.dt.int8, tag="wq")
            nc.sync.dma_start(wq0[:], qru[GROUPS[0]][0])
            nc.scalar.dma_start(b1[:], bp[:])
            nc.scalar.dma_start(xsb[:], xp[:])
            for j in range(3):
                nc.scalar.dma_start(
                    sb[32 * j:32 * j + M, :], sp[M * j:M * (j + 1), :]
                )
            nc.gpsimd.memset(ones[:], 1.0)

            # one PSUM allocation spanning 7 banks: chunk c lives at
            # columns [c*512, (c+1)*512) (bank-aligned), partition rows
            # 32*grp(c) .. +8 — lets the scale-muls read whole group spans
            psum = psp.tile([128, NS], mybir.dt.float32, tag="psum")
            for c in ISSUE:
                lo = GRP_BASE[c]
                # bias row opens the accumulation group: psum = ones^T @ bias
                nc.tensor.matmul(
                    psum[lo:lo + M, c * 512:(c + 1) * 512],
                    ones[:], b1[:, c * 512:(c + 1) * 512],
                    start=True, stop=False,
                )

            kt0 = 0
            for g, gu in enumerate(GROUPS):
                assert kt0 % gu == 0
                if g == 0:
                    wq = wq0
                else:
                    wq = wqp.tile([128, gu, NS], mybir.dt.int8, tag="wq")
                    nc.sync.dma_start(wq[:], qru[gu][kt0 // gu])
                wf = wfp.tile([128, gu, NS], mybir.dt.float16, tag="wf")
                nc.vector.tensor_copy(wf[:, :, 0:DV], wq[:, :, 0:DV])
                nc.scalar.activation(
                    wf[:, :, DV:DV + DA], wq[:, :, DV:DV + DA],
                    mybir.ActivationFunctionType.Copy,
                )
                nc.gpsimd.tensor_copy(wf[:, :, DV + DA:NS], wq[:, :, DV + DA:NS])
                for u in range(gu):
                    kt = kt0 + u
                    last = kt == KT - 1
                    for c in ISSUE:
                        lo = GRP_BASE[c]
                        nc.tensor.matmul(
                            psum[lo:lo + M, c * 512:(c + 1) * 512],
                            xsb[:, kt * M:(kt + 1) * M],
                            wf[:, u, c * 512:(c + 1) * 512],
                            start=False, stop=last,
                        )
                kt0 += gu

            # tail: one scale-multiply per PE column group on VectorE
            # (Pool/GpSimd cannot access PSUM), each followed by its slice
            # of the output DMA on the sync queue.
            osb = outp.tile([72, NS], mybir.dt.float16, tag="osb")
            for j in (0, 1, 2):
                nlo, nhi = GRP_SPAN[j]
                plo = 32 * j
                nc.vector.tensor_mul(
                    osb[plo:plo + M, nlo:nhi],
                    psum[plo:plo + M, nlo:nhi],
                    sb[plo:plo + M, nlo:nhi],
                )
                nc.sync.dma_start(op[:, nlo:nhi], osb[plo:plo + M, nlo:nhi])

    nc.compile()
    return nc


def _get_nc():
    if "nc" not in _CACHE:
        _CACHE["nc"] = _build()
    return _CACHE["nc"]


def _prep_inputs(x, qweight, w_scales, bias):
    x2 = np.asarray(x, dtype=np.float16).reshape(M, K)
    # xsb[p, kt*M + m] = x[m, kt*128 + p]
    xsb = np.ascontiguousarray(
        x2.T.reshape(KT, 128, M).transpose(1, 0, 2).reshape(128, KT * M)
    )
    qweight = np.asarray(qweight)
    w_scales = np.asarray(w_scales, dtype=np.float16).reshape(N)
    bias = np.asarray(bias, dtype=np.float16).reshape(N)
    in_maps = []
    for c in range(NCORES):
        sl = slice(c * NS, (c + 1) * NS)
        qt = qweight[sl, :].T                                 # [K, NS] int8
        # k-major: qh[p, kt*NS + n] = qt[kt*128 + p, n]
        qh = np.ascontiguousarray(
            qt.reshape(KT, 128, NS).transpose(1, 0, 2).reshape(128, KT * NS)
        )
        sc = np.broadcast_to(w_scales[sl], (3 * M, NS))
        sc = np.ascontiguousarray(sc)                          # [24, NS] fp16
        # bias enters the PSUM accumulation before the scale multiply, so
        # pre-divide: out = (sum x*q + b/s) * s
        bos = (bias[sl].astype(np.float32)
               / w_scales[sl].astype(np.float32)).astype(np.float16)
        b1 = np.ascontiguousarray(bos.reshape(1, NS))          # [1, NS] fp16
        in_maps.append({"x": xsb, "qt": qh, "s": sc, "b": b1})
    return in_maps


def _run(x, qweight, w_scales, bias, trace=False):
    nc = _get_nc()
    in_maps = _prep_inputs(x, qweight, w_scales, bias)
    res = run_bass_kernel_spmd(
        nc, in_maps, core_ids=list(range(NCORES)), trace=trace
    )
    y = np.concatenate(
        [np.asarray(res.results[c]["out"]) for c in range(NCORES)], axis=1
    )
    return y.reshape(B, S, N).astype(np.float16), res


def kernel(x, qweight, w_scales, bias):
    y, _ = _run(x, qweight, w_scales, bias, trace=False)
    return y


def kernel_traced(x, qweight, w_scales, bias):
    """Like kernel() but also returns the BassKernelResults (exec_time_ns)."""
    return _run(x, qweight, w_scales, bias, trace=True)


# revision 5
# speedup vs baseline: 1.4906x; 1.4906x over previous
"""W8A16 column-parallel linear for TRN2, 8 NeuronCores.

Computes y = x @ (qweight * w_scales).T + bias with
  x        [8, 1, 8192]  fp16
  qweight  [28672, 8192] int8 (per-row symmetric quant)
  w_scales [28672, 1]    fp16
  bias     [28672]       fp16
  y        [8, 1, 28672] fp16

Sharding: column-parallel — each of the 8 cores owns 3584 output rows
(qweight/w_scales/bias shard), x replicated. No collectives; outputs are
concatenated on the host.

Per-core kernel: stream the int8 weight shard from HBM in grouped
k-tiles (host pre-permuted to k-major so each partition's group slice is
one contiguous run -> large DMA descriptors, ~430GB/s), convert
int8->fp16 on-chip (free-dim split: VectorE 2x-port CAST on [0,DV),
ScalarE activation Copy on [DV,NS)), and accumulate [8, 512]-chunk PSUM
regions with fp16 matmuls (stationary x^T tile, moving weight tile; 7
chunks spread over 3 PE column groups so three moving streams run
concurrently). The conversion pair is the pipeline pole (~2.99
f-elems/ns vs DMA ~3.4), so one 4-ktile group ships as raw fp16
(exact int8 values, 2B/elem) straight into a wf tile - no conversion -
which rebalances converter vs DMA load. Head groups ramp [1,1,2] so the
first conversion starts as soon as possible. Bias/scales enter as
out = (sum x*q + b/s) * s: a K=1 matmul of ones^T @ (b/s) opens each
PSUM accumulation group. Tail: VectorE scale-muls span 0 from PSUM
while ScalarE stages spans 1,2 PSUM->SBUF fp16, then VectorE finishes
them with 2x-port fp16 muls. x/bias/scales ride the Activation-engine
DGE queue so the sync-engine HWDGE queue carries nothing but weights.
"""

import numpy as np

import concourse.bacc as bacc
import concourse.mybir as mybir
import concourse.tile as tile
from concourse.bass_utils import run_bass_kernel_spmd

B, S, K, N = 8, 1, 8192, 28672
M = B * S                 # 8 rows in the GEMM
NCORES = 8
NS = N // NCORES          # 3584 output rows per core
KT = K // 128             # 64 k-tiles
NCHUNK = NS // 512        # 7 psum chunks of 512

# free-dim split of the int8->fp16 conversion:
# VectorE takes [0, DV), ScalarE the rest
DV = 2240

# k-tiles shipped as raw fp16 (no conversion); the rest ship int8
F16_KT = [40, 41, 42, 43]
INT8_KT = [kt for kt in range(KT) if kt not in F16_KT]   # 60 k-tiles

# int8 group sizes (in k-tiles) in stream order; None marks where the
# fp16 group slots into the stream. Head ramps [1,1,2] so conversion
# starts ~3us earlier; tail [2,1,1] drains fast.
GROUPS = [1, 1, 2] + [4] * 9 + [None] + [4] * 4 + [2, 1, 1]
assert sum(g for g in GROUPS if g) == 60

_CACHE = {}

# chunk -> PE column-group (0,1,2 -> array cols 0-31/32-63/64-95). Three
# concurrent moving streams triple the PE's weight-streaming rate.
CHUNK_GRP = [0, 0, 0, 1, 1, 2, 2]
GRP_BASE = [32 * j for j in CHUNK_GRP]         # PSUM base partition per chunk
GRP_SPAN = {0: (0, 1536), 1: (1536, 2560), 2: (2560, 3584)}


def _build():
    nc = bacc.Bacc()
    xp = nc.declare_dram_parameter("x", [128, KT * M], mybir.dt.float16, isOutput=False)
    # k-major int8 weights, ktiles packed in stream order:
    # qp[p, j*NS + n] = w[n, INT8_KT[j]*128 + p]
    qp = nc.declare_dram_parameter("qt", [128, 60 * NS], mybir.dt.int8, isOutput=False)
    # raw fp16 weights for F16_KT (exact int8 values)
    qfp = nc.declare_dram_parameter("qf", [128, 4 * NS], mybir.dt.float16, isOutput=False)
    sp = nc.declare_dram_parameter("s", [24, NS], mybir.dt.float16, isOutput=False)
    bp = nc.declare_dram_parameter("b", [1, NS], mybir.dt.float16, isOutput=False)
    op = nc.declare_dram_parameter("out", [M, NS], mybir.dt.float16, isOutput=True)

    # whole-param rearranges per group size: emits the efficient DMA
    # descriptor layout (one contiguous usz*NS run per partition)
    qru = {
        usz: qp.rearrange("p (g u n) -> g p u n", u=usz, n=NS)
        for usz in (1, 2, 4)
    }

    # per-ktile matmul issue order rotates through the PE column groups so
    # the three streams start back-to-back instead of blocking each other
    ISSUE = [0, 3, 5, 1, 4, 6, 2]

    with tile.TileContext(nc) as tc:
        with (
            tc.tile_pool(name="const", bufs=1) as constp,
            tc.tile_pool(name="wq", bufs=5) as wqp,
            tc.tile_pool(name="wf", bufs=3) as wfp,
            tc.tile_pool(name="psum", bufs=1, space="PSUM") as psp,
            tc.tile_pool(name="outp", bufs=1) as outp,
        ):
            xsb = constp.tile([128, KT * M], mybir.dt.float16, tag="xsb")
            sb = constp.tile([72, NS], mybir.dt.float16, tag="sb")
            b1 = constp.tile([1, NS], mybir.dt.float16, tag="b1")
            ones = constp.tile([1, M], mybir.dt.float16, tag="ones")
            # fp16 staging for tail spans 1,2 (partition-aligned to psum)
            stg = constp.tile([72, 2048], mybir.dt.float16, tag="stg")

            # weight stream owns the sync-engine HWDGE queue exclusively;
            # constants ride the Activation-engine DGE queue in parallel
            wq0 = wqp.tile([128, GROUPS[0], NS], mybir.dt.int8, tag="wq")
            nc.sync.dma_start(wq0[:], qru[GROUPS[0]][0])
            nc.scalar.dma_start(b1[:], bp[:])
            nc.scalar.dma_start(xsb[:], xp[:])
            for j in range(3):
                nc.scalar.dma_start(
                    sb[32 * j:32 * j + M, :], sp[M * j:M * (j + 1), :]
                )
            nc.gpsimd.memset(ones[:], 1.0)

            # one PSUM allocation spanning 7 banks: chunk c lives at
            # columns [c*512, (c+1)*512) (bank-aligned), partition rows
            # 32*grp(c) .. +8 — lets the tail ops read whole group spans
            psum = psp.tile([128, NS], mybir.dt.float32, tag="psum")
            for c in ISSUE:
                lo = GRP_BASE[c]
                # bias row opens the accumulation group: psum = ones^T @ bias
                nc.tensor.matmul(
                    psum[lo:lo + M, c * 512:(c + 1) * 512],
                    ones[:], b1[:, c * 512:(c + 1) * 512],
                    start=True, stop=False,
                )

            def mm(wf_tile, u, kt, last):
                for c in ISSUE:
                    lo = GRP_BASE[c]
                    nc.tensor.matmul(
                        psum[lo:lo + M, c * 512:(c + 1) * 512],
                        xsb[:, kt * M:(kt + 1) * M],
                        wf_tile[:, u, c * 512:(c + 1) * 512],
                        start=False, stop=last,
                    )

            j0 = 0
            for g, gu in enumerate(GROUPS):
                if gu is None:
                    # fp16 group: DMA straight into a wf-pool tile, no
                    # conversion; matmuls read it directly
                    wf = wfp.tile([128, 4, NS], mybir.dt.float16, tag="wf")
                    nc.sync.dma_start(wf[:], qfp.rearrange("p (u n) -> p u n", n=NS))
                    for u, kt in enumerate(F16_KT):
                        mm(wf, u, kt, last=False)
                    continue
                if g == 0:
                    wq = wq0
                else:
                    wq = wqp.tile([128, gu, NS], mybir.dt.int8, tag="wq")
                    nc.sync.dma_start(wq[:], qru[gu][j0 // gu])
                wf = wfp.tile([128, gu, NS], mybir.dt.float16, tag="wf")
                nc.vector.tensor_copy(wf[:, :, 0:DV], wq[:, :, 0:DV])
                nc.scalar.activation(
                    wf[:, :, DV:NS], wq[:, :, DV:NS],
                    mybir.ActivationFunctionType.Copy,
                )
                for u in range(gu):
                    kt = INT8_KT[j0 + u]
                    mm(wf, u, kt, last=kt == KT - 1)
                j0 += gu

            # tail: span 0 scale-mul straight from PSUM on VectorE while
            # ScalarE stages spans 1,2 PSUM->SBUF fp16; VectorE then
            # finishes spans 1,2 with 2x-port all-SBUF fp16 muls. Each
            # span's output DMA issues as soon as its mul lands.
            osb = outp.tile([72, NS], mybir.dt.float16, tag="osb")
            for j in (1, 2):
                nlo, nhi = GRP_SPAN[j]
                plo = 32 * j
                nc.scalar.activation(
                    stg[plo:plo + M, nlo - 1536:nhi - 1536],
                    psum[plo:plo + M, nlo:nhi],
                    mybir.ActivationFunctionType.Copy,
                )
            nc.vector.tensor_mul(
                osb[0:M, 0:1536], psum[0:M, 0:1536], sb[0:M, 0:1536]
            )
            nc.sync.dma_start(op[:, 0:1536], osb[0:M, 0:1536])
            for j in (1, 2):
                nlo, nhi = GRP_SPAN[j]
                plo = 32 * j
                nc.vector.tensor_mul(
                    osb[plo:plo + M, nlo:nhi],
                    stg[plo:plo + M, nlo - 1536:nhi - 1536],
                    sb[plo:plo + M, nlo:nhi],
                )
                nc.sync.dma_start(op[:, nlo:nhi], osb[plo:plo + M, nlo:nhi])

    nc.compile()
    return nc


def _get_nc():
    if "nc" not in _CACHE:
        _CACHE["nc"] = _build()
    return _CACHE["nc"]


def _prep_inputs(x, qweight, w_scales, bias):
    x2 = np.asarray(x, dtype=np.float16).reshape(M, K)
    # xsb[p, kt*M + m] = x[m, kt*128 + p]
    xsb = np.ascontiguousarray(
        x2.T.reshape(KT, 128, M).transpose(1, 0, 2).reshape(128, KT * M)
    )
    qweight = np.asarray(qweight)
    w_scales = np.asarray(w_scales, dtype=np.float16).reshape(N)
    bias = np.asarray(bias, dtype=np.float16).reshape(N)
    in_maps = []
    for c in range(NCORES):
        sl = slice(c * NS, (c + 1) * NS)
        qt = qweight[sl, :].T                                 # [K, NS] int8
        qk = qt.reshape(KT, 128, NS)                          # [KT,128,NS]
        # int8 ktiles in stream order, k-major: qh[p, j*NS+n]
        qh = np.ascontiguousarray(
            qk[INT8_KT].transpose(1, 0, 2).reshape(128, 60 * NS)
        )
        # raw fp16 ktiles (exact int8 values)
        qf = np.ascontiguousarray(
            qk[F16_KT].astype(np.float16).transpose(1, 0, 2).reshape(128, 4 * NS)
        )
        sc = np.ascontiguousarray(np.broadcast_to(w_scales[sl], (3 * M, NS)))
        # bias enters the PSUM accumulation before the scale multiply, so
        # pre-divide: out = (sum x*q + b/s) * s
        bos = (bias[sl].astype(np.float32)
               / w_scales[sl].astype(np.float32)).astype(np.float16)
        b1 = np.ascontiguousarray(bos.reshape(1, NS))          # [1, NS] fp16
        in_maps.append({"x": xsb, "qt": qh, "qf": qf, "s": sc, "b": b1})
    return in_maps


def _run(x, qweight, w_scales, bias, trace=False):
    nc = _get_nc()
    in_maps = _prep_inputs(x, qweight, w_scales, bias)
    res = run_bass_kernel_spmd(
        nc, in_maps, core_ids=list(range(NCORES)), trace=trace
    )
    y = np.concatenate(
        [np.asarray(res.results[c]["out"]) for c in range(NCORES)], axis=1
    )
    return y.reshape(B, S, N).astype(np.float16), res


def kernel(x, qweight, w_scales, bias):
    y, _ = _run(x, qweight, w_scales, bias, trace=False)
    return y


def kernel_traced(x, qweight, w_scales, bias):
    """Like kernel() but also returns the BassKernelResults (exec_time_ns)."""
    return _run(x, qweight, w_scales, bias, trace=True)


# revision 8
# speedup vs baseline: 1.6139x; 1.0827x over previous
"""W8A16 column-parallel linear for TRN2, 8 NeuronCores.

Computes y = x @ (qweight * w_scales).T + bias with
  x        [8, 1, 8192]  fp16
  qweight  [28672, 8192] int8 (per-row symmetric quant)
  w_scales [28672, 1]    fp16
  bias     [28672]       fp16
  y        [8, 1, 28672] fp16

Sharding: column-parallel — each of the 8 cores owns 3584 output rows
(qweight/w_scales/bias shard), x replicated. No collectives; outputs are
concatenated on the host.

Per-core kernel: stream the int8 weight shard from HBM in grouped
k-tiles (host pre-permuted to k-major so each partition's group slice is
one contiguous run -> large DMA descriptors, ~430GB/s), convert
int8->fp16 on-chip (free-dim split: VectorE 2x-port CAST on [0,DV),
ScalarE activation Copy on [DV,NS)), and accumulate [8, 512]-chunk PSUM
regions with fp16 matmuls (stationary x^T tile, moving weight tile).
The 7 chunks spread over FOUR PE column groups (PSUM base partition) so
no column group carries more than 2 chunks — keeps each PE moving
stream (~0.93us/ktile incl LDWEIGHTS) under the conversion cadence
(~1.21us/ktile), which is the pipeline pole. One 4-ktile group ships as
raw fp16 (exact int8 values) into a dedicated prefetched tile — no
conversion — rebalancing converter vs DMA load. Head groups ramp
[1,1,2] so the first conversion starts ~3us early. Bias/scales enter as
out = (sum x*q + b/s) * s: a K=1 matmul of ones^T @ (b/s) opens each
PSUM accumulation group. Tail: VectorE scale-muls span 0 from PSUM
while ScalarE stages spans 1-3 PSUM->SBUF fp16 (in place in the output
tile), then VectorE finishes them with 2x-port fp16 muls. Constants
(x/bias/scales) ride the Pool-engine SWDGE so the sync-engine HWDGE
queue carries nothing but weights.
"""

import numpy as np

import concourse.bacc as bacc
import concourse.mybir as mybir
import concourse.tile as tile
from concourse.bass_utils import run_bass_kernel_spmd

B, S, K, N = 8, 1, 8192, 28672
M = B * S                 # 8 rows in the GEMM
NCORES = 8
NS = N // NCORES          # 3584 output rows per core
KT = K // 128             # 64 k-tiles
NCHUNK = NS // 512        # 7 psum chunks of 512

# free-dim split of the int8->fp16 conversion:
# VectorE takes [0, DV), ScalarE the rest
DV = 2240

# k-tiles shipped as raw fp16 (no conversion); the rest ship int8
F16_KT = [40, 41, 42, 43]
INT8_KT = [kt for kt in range(KT) if kt not in F16_KT]   # 60 k-tiles

# int8 group sizes (in k-tiles) in stream order; None marks where the
# fp16 group slots into the stream. Head ramps [1,1,2] so conversion
# starts ~3us earlier; tail [2,1,1] drains fast.
GROUPS = [1, 1, 2] + [4] * 9 + [None] + [4] * 4 + [2, 1, 1]
assert sum(g for g in GROUPS if g) == 60

_CACHE = {}

# chunk -> PE column-group (0..3 -> array cols 0-31/32-63/64-95/96-127).
# Four concurrent moving streams; max 2 chunks per group keeps each
# stream below the conversion cadence.
CHUNK_GRP = [0, 0, 1, 1, 2, 2, 3]
GRP_BASE = [32 * j for j in CHUNK_GRP]         # PSUM base partition per chunk
GRP_SPAN = {0: (0, 1024), 1: (1024, 2048), 2: (2048, 3072), 3: (3072, 3584)}


def _build():
    nc = bacc.Bacc()
    xp = nc.declare_dram_parameter("x", [128, KT * M], mybir.dt.float16, isOutput=False)
    # k-major int8 weights, ktiles packed in stream order:
    # qp[p, j*NS + n] = w[n, INT8_KT[j]*128 + p]
    qp = nc.declare_dram_parameter("qt", [128, 60 * NS], mybir.dt.int8, isOutput=False)
    # raw fp16 weights for F16_KT (exact int8 values)
    qfp = nc.declare_dram_parameter("qf", [128, 4 * NS], mybir.dt.float16, isOutput=False)
    sp = nc.declare_dram_parameter("s", [32, NS], mybir.dt.float16, isOutput=False)
    bp = nc.declare_dram_parameter("b", [1, NS], mybir.dt.float16, isOutput=False)
    op = nc.declare_dram_parameter("out", [M, NS], mybir.dt.float16, isOutput=True)

    # whole-param rearranges per group size: emits the efficient DMA
    # descriptor layout (one contiguous usz*NS run per partition)
    qru = {
        usz: qp.rearrange("p (g u n) -> g p u n", u=usz, n=NS)
        for usz in (1, 2, 4)
    }

    # per-ktile matmul issue order rotates through the PE column groups so
    # adjacent issues land on different groups and stream concurrently
    ISSUE = [0, 2, 4, 6, 1, 3, 5]

    with tile.TileContext(nc) as tc:
        with (
            tc.tile_pool(name="const", bufs=1) as constp,
            tc.tile_pool(name="wq", bufs=5) as wqp,
            tc.tile_pool(name="wf", bufs=3) as wfp,
            tc.tile_pool(name="psum", bufs=1, space="PSUM") as psp,
            tc.tile_pool(name="outp", bufs=1) as outp,
        ):
            xsb = constp.tile([128, KT * M], mybir.dt.float16, tag="xsb")
            # scales on rows 32j..32j+8 (psum-partition-aligned)
            sb = constp.tile([104, NS], mybir.dt.float16, tag="sb")
            b1t = constp.tile([1, NS], mybir.dt.float16, tag="b1")
            ones = constp.tile([1, M], mybir.dt.float16, tag="ones")
            wff = constp.tile([128, 4, NS], mybir.dt.float16, tag="wff")
            b1 = b1t[:]

            # weight stream owns the sync-engine HWDGE queue exclusively;
            # constants ride the Pool-engine SWDGE in parallel
            wq0 = wqp.tile([128, GROUPS[0], NS], mybir.dt.int8, tag="wq")
            nc.sync.dma_start(wq0[:], qru[GROUPS[0]][0])
            nc.gpsimd.dma_start(b1, bp[:])
            nc.gpsimd.dma_start(xsb[:], xp[:])
            for j in range(4):
                nc.gpsimd.dma_start(
                    sb[32 * j:32 * j + M, :], sp[M * j:M * (j + 1), :]
                )
            nc.gpsimd.memset(ones[:], 1.0)

            # one PSUM allocation spanning 7 banks: chunk c lives at
            # columns [c*512, (c+1)*512) (bank-aligned), partition rows
            # 32*grp(c) .. +8 — lets the tail ops read whole group spans
            psum = psp.tile([128, NS], mybir.dt.float32, tag="psum")
            for c in ISSUE:
                lo = GRP_BASE[c]
                # bias row opens the accumulation group: psum = ones^T @ bias
                nc.tensor.matmul(
                    psum[lo:lo + M, c * 512:(c + 1) * 512],
                    ones[:], b1[:, c * 512:(c + 1) * 512],
                    start=True, stop=False, tile_position=(0, lo),
                )

            def mm(wf_tile, u, kt, last):
                for c in ISSUE:
                    lo = GRP_BASE[c]
                    nc.tensor.matmul(
                        psum[lo:lo + M, c * 512:(c + 1) * 512],
                        xsb[:, kt * M:(kt + 1) * M],
                        wf_tile[:, u, c * 512:(c + 1) * 512],
                        start=False, stop=last, tile_position=(0, lo),
                    )

            j0 = 0
            for g, gu in enumerate(GROUPS):
                if gu is None:
                    # fp16 group: dedicated tile, DMA'd in stream position
                    # on the weight queue; matmuls read it directly
                    nc.sync.dma_start(
                        wff[:], qfp.rearrange("p (u n) -> p u n", n=NS)
                    )
                    for u, kt in enumerate(F16_KT):
                        mm(wff, u, kt, last=False)
                    continue
                if g == 0:
                    wq = wq0
                else:
                    wq = wqp.tile([128, gu, NS], mybir.dt.int8, tag="wq")
                    nc.sync.dma_start(wq[:], qru[gu][j0 // gu])
                wf = wfp.tile([128, gu, NS], mybir.dt.float16, tag="wf")
                nc.vector.tensor_copy(wf[:, :, 0:DV], wq[:, :, 0:DV])
                nc.scalar.activation(
                    wf[:, :, DV:NS], wq[:, :, DV:NS],
                    mybir.ActivationFunctionType.Copy,
                )
                for u in range(gu):
                    kt = INT8_KT[j0 + u]
                    mm(wf, u, kt, last=kt == KT - 1)
                j0 += gu

            # tail: span 0 scale-mul straight from PSUM on VectorE while
            # ScalarE stages spans 1-3 PSUM->SBUF fp16 in place in osb;
            # VectorE then finishes them with 2x-port all-SBUF fp16 muls.
            # Each span's output DMA issues as soon as its mul lands.
            osb = outp.tile([104, NS], mybir.dt.float16, tag="osb")
            nc.vector.tensor_mul(
                osb[0:M, 0:1024], psum[0:M, 0:1024], sb[0:M, 0:1024]
            )
            for j in (1, 2, 3):
                nlo, nhi = GRP_SPAN[j]
                plo = 32 * j
                nc.scalar.activation(
                    osb[plo:plo + M, nlo:nhi],
                    psum[plo:plo + M, nlo:nhi],
                    mybir.ActivationFunctionType.Copy,
                )
            nc.sync.dma_start(op[:, 0:1024], osb[0:M, 0:1024])
            for j in (1, 2, 3):
                nlo, nhi = GRP_SPAN[j]
                plo = 32 * j
                nc.vector.tensor_mul(
                    osb[plo:plo + M, nlo:nhi],
                    osb[plo:plo + M, nlo:nhi],
                    sb[plo:plo + M, nlo:nhi],
                )
                nc.sync.dma_start(op[:, nlo:nhi], osb[plo:plo + M, nlo:nhi])

    nc.compile()
    return nc


def _get_nc():
    if "nc" not in _CACHE:
        _CACHE["nc"] = _build()
    return _CACHE["nc"]


def _prep_inputs(x, qweight, w_scales, bias):
    x2 = np.asarray(x, dtype=np.float16).reshape(M, K)
    # xsb[p, kt*M + m] = x[m, kt*128 + p]
    xsb = np.ascontiguousarray(
        x2.T.reshape(KT, 128, M).transpose(1, 0, 2).reshape(128, KT * M)
    )
    qweight = np.asarray(qweight)
    w_scales = np.asarray(w_scales, dtype=np.float16).reshape(N)
    bias = np.asarray(bias, dtype=np.float16).reshape(N)
    in_maps = []
    for c in range(NCORES):
        sl = slice(c * NS, (c + 1) * NS)
        qt = qweight[sl, :].T                                 # [K, NS] int8
        qk = qt.reshape(KT, 128, NS)                          # [KT,128,NS]
        # int8 ktiles in stream order, k-major: qh[p, j*NS+n]
        qh = np.ascontiguousarray(
            qk[INT8_KT].transpose(1, 0, 2).reshape(128, 60 * NS)
        )
        # raw fp16 ktiles (exact int8 values)
        qf = np.ascontiguousarray(
            qk[F16_KT].astype(np.float16).transpose(1, 0, 2).reshape(128, 4 * NS)
        )
        sc = np.ascontiguousarray(np.broadcast_to(w_scales[sl], (4 * M, NS)))
        # bias enters the PSUM accumulation before the scale multiply, so
        # pre-divide: out = (sum x*q + b/s) * s
        bos = (bias[sl].astype(np.float32)
               / w_scales[sl].astype(np.float32)).astype(np.float16)
        b1 = np.ascontiguousarray(bos.reshape(1, NS))          # [1, NS] fp16
        in_maps.append({"x": xsb, "qt": qh, "qf": qf, "s": sc, "b": b1})
    return in_maps


def _run(x, qweight, w_scales, bias, trace=False):
    nc = _get_nc()
    in_maps = _prep_inputs(x, qweight, w_scales, bias)
    res = run_bass_kernel_spmd(
        nc, in_maps, core_ids=list(range(NCORES)), trace=trace
    )
    y = np.concatenate(
        [np.asarray(res.results[c]["out"]) for c in range(NCORES)], axis=1
    )
    return y.reshape(B, S, N).astype(np.float16), res


def kernel(x, qweight, w_scales, bias):
    y, _ = _run(x, qweight, w_scales, bias, trace=False)
    return y


def kernel_traced(x, qweight, w_scales, bias):
    """Like kernel() but also returns the BassKernelResults (exec_time_ns)."""
    return _run(x, qweight, w_scales, bias, trace=True)
